# revision 27
# baseline (speedup 1.0000x reference)
"""Trainium2 Bass kernel for nn_MultiHeadAttention (B=2, S=2048, D=1024, H=16, dk=64).

Sharding: 8 cores = (batch b in {0,1}) x (head group g in {0..3}, 4 heads each).
The reference's RAW reshape (B,H,S,dk) -> (B,S,H*dk) means output row
s' = h*128 + s//16 depends only on head h, so core (b,g) produces output rows
[512g, 512(g+1)) of batch b -- pure concatenation, no collectives.

v2 (fp16 + software pipeline):
  - All matmul operands fp16 (1 PE column/cycle vs 4-pass fp32 HIGH mode);
    only the exp outputs are bf16 (they need the exponent range).  PSUM fp32.
  - q columns are permuted host-side (j-major within each 512 block) so the
    normalize write into the head/seq-mixed hr layout is contiguous.
  - Scores pack 2 heads in the PE array via row tile_position (concurrent).
  - Softmax denominator rides as a ones column in the V operand; reciprocal
    via copy-to-SBUF + reciprocal_approx_fast (direct-from-PSUM is broken).
  - The kernel is one long software pipeline: K/Q/V projections, the output
    (WO) matmuls, and all DMAs are interleaved as "filler" into the
    exp-paced attention loop so the tensor engine stays continuously busy
    (HAM up-clocks only after ~4us without gaps).  kT/qT stream tiles stay
    resident so the second head pair's projections need no re-DMA.
  - Optionally (K_SCHRAUD=1) one of 8 kp slots' exps is computed on the DVE
    with a Schraudolph int16/bf16-bits approximation to relieve the ACT
    engine (softmax normalization cancels most of the ~2% weight error).
"""

import sys

try:
    import concourse.bass as bass  # noqa: F401
except ImportError:
    sys.path.insert(0, "/opt/trn_rl_repo")

import os

import numpy as np

FILLER = os.environ.get("K_FILLER", "1") == "1"
# "approx": copy denom row to SBUF then reciprocal_approx_fast (fast path;
# approx direct from PSUM returned garbage on HW). "exact": nc.vector.reciprocal.
RECIP_MODE = os.environ.get("K_RECIP", "approx")
# comma-separated kp slots whose exp runs on DVE (Schraudolph) instead of ACT
SCHRAUD_KPS = frozenset(
    int(x) for x in os.environ.get("K_SCHRAUD_KPS", "3,6").split(",") if x != ""
)
# per-kp split instead: ACT takes head A + half of head B, DVE-Schraudolph
# takes the other half of head B every kp (overrides SCHRAUD_KPS)
EXP_SPLIT = os.environ.get("K_EXP_SPLIT", "0") == "1"

import concourse.bacc as bacc
import concourse.tile as tile
from concourse import mybir
from concourse.bass_utils import run_bass_kernel_spmd

BF = mybir.dt.bfloat16
I16 = mybir.dt.int16
F16 = mybir.dt.float16
F32 = mybir.dt.float32

B, S, D, H, DK = 2, 2048, 1024, 16, 64
GROUPS = 4
SCALE = 1.0 / 8.0  # 1/sqrt(dk)
# Schraudolph exp in bf16 bit space: bits = round(x*SCALE*(2^7/ln2) + B0)
SCH_A = 128.0 / float(np.log(2.0)) * SCALE
SCH_B = 16250.5

_cached_nc = None


def build_nc():
    nc = bacc.Bacc(None, target_bir_lowering=False)
    qT = nc.dram_tensor("qT", [D, S], F16, kind="ExternalInput")
    kT = nc.dram_tensor("kT", [D, S], F16, kind="ExternalInput")
    vT = nc.dram_tensor("vT", [D, S], F16, kind="ExternalInput")
    wq = nc.dram_tensor("wq", [D, 256], F16, kind="ExternalInput")
    wk = nc.dram_tensor("wk", [D, 256], F16, kind="ExternalInput")
    wv = nc.dram_tensor("wv", [D, 256], F16, kind="ExternalInput")
    wo = nc.dram_tensor("wo", [D, D], F16, kind="ExternalInput")
    out = nc.dram_tensor("out", [512, D], F32, kind="ExternalOutput")

    Exp = mybir.ActivationFunctionType.Exp
    Mult = mybir.AluOpType.mult
    Add = mybir.AluOpType.add

    with tile.TileContext(nc) as tc, nc.allow_low_precision(
        reason="fp16/bf16 matmuls with fp32 PSUM accumulation; attention "
        "weight and normalization rounding averages out over 2048 positions"
    ):
        with (
            tc.tile_pool(name="persist", bufs=1) as persist,
            tc.tile_pool(name="wqkv", bufs=1) as wqkv,
            tc.tile_pool(name="hrp", bufs=4) as hrp,
            tc.tile_pool(name="xhp", bufs=2) as xhp,
            tc.tile_pool(name="small", bufs=4) as small,
            tc.tile_pool(name="opool", bufs=2) as opool,
            tc.tile_pool(name="epool", bufs=10) as epool,
            tc.tile_pool(name="kstream", bufs=4) as kstream,
            tc.tile_pool(name="qstream", bufs=4) as qstream,
            tc.tile_pool(name="vstream", bufs=2) as vstream,
            tc.tile_pool(name="ps_mix", bufs=1, space="PSUM") as ps_mix,
            tc.tile_pool(name="ps_sc", bufs=2, space="PSUM") as ps_sc,
            tc.tile_pool(name="ps_pv", bufs=3, space="PSUM") as ps_pv,
        ):
            qpT = persist.tile([128, 2, S], F16, tag="qpT")
            kpT = persist.tile([128, 2, S], F16, tag="kpT")
            vaug = persist.tile([128, 16, 4, 65], BF, tag="vaug")
            ones_f32 = persist.tile([128, 1], F32, tag="ones_f32")
            nc.vector.memset(ones_f32, 1.0)
            nc.vector.tensor_copy(
                vaug[:, :, :, 64:65], ones_f32.to_broadcast((128, 16, 4, 1))
            )
            ones_bf = persist.tile([1, 64], BF, tag="ones_bf")
            nc.vector.tensor_copy(ones_bf, ones_f32[0:1, :].to_broadcast((1, 64)))

            wq_sb = wqkv.tile([128, 8, 256], F16, tag="wq")
            wk_sb = wqkv.tile([128, 8, 256], F16, tag="wk")
            wv_sb = wqkv.tile([128, 8, 256], F16, tag="wv")
            wo_sb = wqkv.tile([128, 8, D], F16, tag="wo")

            # ---------------- DMA + matmul emission helpers ----------------
            kst, qst, v_st = {}, {}, {}

            def dma_stream(cache, pool, x_dram, nb, tag, eng=None):
                if nb in cache:
                    return cache[nb]
                st = pool.tile([128, 8, 512], F16, tag=tag, name=f"{tag}{nb}")
                (eng or nc.sync).dma_start(
                    out=st,
                    in_=x_dram.rearrange("(t p) s -> p t s", p=128)[
                        :, :, 512 * nb : 512 * (nb + 1)
                    ],
                )
                cache[nb] = st
                return st

            def mm_qk(cache, w_sb, outt, m, nb):
                """One [128, 512] block of a Q/K projection for head pair m
                (stream tile must already be DMA'd)."""
                st = cache[nb]
                ps = ps_mix.tile([128, 512], F32, tag="mix", name="psq")
                for k in range(8):
                    nc.tensor.matmul(
                        ps,
                        w_sb[:, k, 128 * m : 128 * (m + 1)],
                        st[:, k, :],
                        start=(k == 0),
                        stop=(k == 7),
                    )
                nc.vector.tensor_copy(outt[:, m, 512 * nb : 512 * (nb + 1)], ps)

            def emit_v_group(kt):
                """V projection for one 128-row kpos chunk kt (all 4 heads)."""
                st = dma_stream(v_st, vstream, vT, kt // 4, "vst")
                sti = kt % 4
                ps_full = ps_mix.tile([128, 512], F32, tag="mix", name="vps")
                ps = ps_full[:, :256]
                for k in range(8):
                    nc.tensor.matmul(
                        ps,
                        st[:, k, 128 * sti : 128 * (sti + 1)],
                        wv_sb[:, k, :],
                        start=(k == 0),
                        stop=(k == 7),
                    )
                nc.vector.tensor_copy(
                    vaug[:, kt, :, 0:64], ps.rearrange("p (h c) -> p h c", h=4)
                )

            def emit_wo_n(h, xh, n):
                """Half of head h's output projection (one 512-col block).
                The PSUM->SBUF copy runs on ACT (idle by then; Copy shares
                the exp table set so no table reload)."""
                wops = ps_mix.tile([128, 512], F32, tag="mix", name=f"wops{h}")
                for t in range(8):
                    nc.tensor.matmul(
                        wops,
                        xh[:, t, :],
                        wo_sb[:, t, 512 * n : 512 * (n + 1)],
                        start=(t == 0),
                        stop=(t == 7),
                    )
                ot = opool.tile([128, 512], F32, tag="o", name=f"ot{h}")
                nc.scalar.copy(ot, wops)
                nc.sync.dma_start(
                    out=out[128 * h : 128 * (h + 1), 512 * n : 512 * (n + 1)],
                    in_=ot,
                )

            def emit_scatter(h, hr_h):
                """hr (head-transposed) -> X^T chunks for the WO lhsT."""
                xh = xhp.tile([128, 8, 128], F16, tag="xh", name=f"xh{h}")
                hv = hr_h.rearrange("p (j r) -> p j r", j=16)
                for par in range(2):
                    nc.sync.dma_start(
                        out=xh[64 * par : 64 * (par + 1)], in_=hv[:, par::2, :]
                    )
                return xh

            # ---------------- pre-phase ----------------
            # DMA ladder spread across three issue engines (per-engine DMA
            # queues serialize their transfers; parallel queues overlap them).
            # kT/qT tiles stay resident (reused for head pair 1), vT tiles
            # are consumed within (0,0).
            nc.scalar.dma_start(
                out=wk_sb, in_=wk.rearrange("(t p) n -> p t n", p=128)
            )
            nc.gpsimd.dma_start(
                out=wq_sb, in_=wq.rearrange("(t p) n -> p t n", p=128)
            )
            dma_stream(kst, kstream, kT, 0, "kst")  # sync queue
            dma_stream(qst, qstream, qT, 0, "qst", eng=nc.scalar)
            dma_stream(v_st, vstream, vT, 0, "vst", eng=nc.gpsimd)
            nc.scalar.dma_start(
                out=wv_sb, in_=wv.rearrange("(t p) n -> p t n", p=128)
            )
            mm_qk(kst, wk_sb, kpT, 0, 0)
            mm_qk(qst, wq_sb, qpT, 0, 0)
            dma_stream(kst, kstream, kT, 1, "kst")
            dma_stream(kst, kstream, kT, 2, "kst")
            dma_stream(v_st, vstream, vT, 1, "vst", eng=nc.scalar)
            dma_stream(kst, kstream, kT, 3, "kst")
            dma_stream(v_st, vstream, vT, 2, "vst")

            if not FILLER:
                for nb in range(1, 4):
                    mm_qk(kst, wk_sb, kpT, 0, nb)
                for kt in range(16):
                    emit_v_group(kt)
                for nb in range(1, 4):
                    dma_stream(qst, qstream, qT, nb, "qst")
                    mm_qk(qst, wq_sb, qpT, 0, nb)
                for nb in range(4):
                    mm_qk(kst, wk_sb, kpT, 1, nb)
                for nb in range(4):
                    mm_qk(qst, wq_sb, qpT, 1, nb)
                nc.sync.dma_start(
                    out=wo_sb, in_=wo.rearrange("(t p) n -> p t n", p=128)
                )

            # filler plan: (hp, qb) -> list of closures, pulled one per kp
            # slot between the scores and P@V emissions.  RAW rule: anything
            # writing qpT/kpT block X is emitted strictly before the block
            # whose scores read X.  (0,0) has its own denser inline plan.
            filler = {
                (0, 0): [],
                (0, 1): [
                    lambda: (
                        dma_stream(qst, qstream, qT, 2, "qst"),
                        dma_stream(qst, qstream, qT, 3, "qst"),
                        nc.sync.dma_start(
                            out=wo_sb,
                            in_=wo.rearrange("(t p) n -> p t n", p=128),
                        ),
                        mm_qk(kst, wk_sb, kpT, 1, 0),
                    ),
                    lambda: mm_qk(kst, wk_sb, kpT, 1, 1),
                    lambda: mm_qk(kst, wk_sb, kpT, 1, 2),
                    lambda: mm_qk(kst, wk_sb, kpT, 1, 3),
                    lambda: mm_qk(qst, wq_sb, qpT, 0, 2),
                    lambda: mm_qk(qst, wq_sb, qpT, 0, 3),
                ],
                (0, 2): [
                    lambda: mm_qk(qst, wq_sb, qpT, 1, 0),
                    lambda: mm_qk(qst, wq_sb, qpT, 1, 1),
                ],
                (0, 3): [lambda: mm_qk(qst, wq_sb, qpT, 1, 2)],
                (1, 0): [lambda: mm_qk(qst, wq_sb, qpT, 1, 3)],
                (1, 1): [],
                (1, 2): [],  # WO h0 inserted dynamically
                (1, 3): [],  # WO h1 inserted dynamically
            }

            # (0,0) inline slot plan: JIT K-projections and V-projections so
            # attention starts as soon as kT block 0 + qT block 0 land.
            def slot00(kp):
                if kp < 3:
                    mm_qk(kst, wk_sb, kpT, 0, kp + 1)
                if kp == 2:
                    dma_stream(v_st, vstream, vT, 3, "vst")
                if kp == 5:
                    dma_stream(qst, qstream, qT, 1, "qst")
                emit_v_group(2 * kp)
                emit_v_group(2 * kp + 1)
                if kp == 7:
                    mm_qk(qst, wq_sb, qpT, 0, 1)

            hr = {}
            xh_done = {}

            for hp in range(2):
                hA, hB = 2 * hp, 2 * hp + 1
                for h in (hA, hB):
                    hr[h] = hrp.tile([64, 2048], F16, tag="hr", name=f"hr{h}")
                for qb in range(4):
                    fq = list(filler[(hp, qb)])
                    pv = {
                        h: ps_pv.tile([65, 512], F32, tag="pv", name=f"pv{h}")
                        for h in (hA, hB)
                    }
                    e_q = {}

                    def emit_sc(kp, hp=hp, qb=qb, hA=hA, hB=hB, e_q=None):
                        sc = {
                            h: ps_sc.tile([128, 1024], F32, tag="sc",
                                          name=f"sc{h}")
                            for h in (hA, hB)
                        }
                        for half in range(2):
                            kt = 2 * kp + half
                            for i, h in enumerate((hA, hB)):
                                nc.tensor.matmul(
                                    sc[h][:, 512 * half : 512 * (half + 1)],
                                    kpT[64 * i : 64 * (i + 1), hp,
                                        128 * kt : 128 * (kt + 1)],
                                    qpT[64 * i : 64 * (i + 1), hp,
                                        512 * qb : 512 * (qb + 1)],
                                    start=True,
                                    stop=True,
                                    tile_position=(64 * i, 0),
                                )
                        for h in (hA, hB):
                            et = epool.tile([128, 1024], BF, tag="e",
                                            name=f"e{h}")
                            if kp in SCHRAUD_KPS:
                                # Schraudolph exp on DVE: bf16 bits via int16
                                nc.vector.tensor_scalar(
                                    et.bitcast(I16), sc[h], SCH_A, SCH_B,
                                    Mult, Add,
                                )
                            else:
                                nc.scalar.activation(et, sc[h], Exp,
                                                     scale=SCALE)
                            e_q[(kp, h)] = et

                    def emit_pv(kp, pv=pv, hA=hA, hB=hB, e_q=None):
                        for half in range(2):
                            kt = 2 * kp + half
                            for h in (hA, hB):
                                nc.tensor.matmul(
                                    pv[h],
                                    vaug[:, kt, h, :],
                                    e_q[(kp, h)][:,
                                                 512 * half : 512 * (half + 1)],
                                    start=(kt == 0),
                                    stop=(kt == 15),
                                )

                    # sc runs SKEW blocks ahead of pv so the PE never parks
                    # on the pv accumulation right at a qb boundary (the
                    # normalize chain of the previous qb is still draining).
                    SKEW = 2
                    for kp in range(8):
                        emit_sc(kp, e_q=e_q)
                        # filler between scores and P@V, where PE waits on ACT
                        if FILLER:
                            if hp == 0 and qb == 0:
                                slot00(kp)
                            elif fq:
                                fq.pop(0)()
                        if kp >= SKEW:
                            emit_pv(kp - SKEW, e_q=e_q)
                    for kp in range(8 - SKEW, 8):
                        emit_pv(kp, e_q=e_q)
                    # normalize + head/seq-mixed layout write.  q columns were
                    # permuted host-side to j-major within each 512 block, so
                    # both the pv read and the hr write are contiguous runs.
                    for h in (hA, hB):
                        rc = small.tile([1, 512], F32, tag="rc", name=f"rc{h}")
                        if RECIP_MODE == "exact":
                            nc.vector.reciprocal(rc, pv[h][64:65, :])
                        else:
                            dn = small.tile([1, 512], F32, tag="dn",
                                            name=f"dn{h}")
                            nc.vector.tensor_copy(dn, pv[h][64:65, :])
                            nc.vector.reciprocal_approx_fast(rc, dn)
                        rcb = small.tile([1, 512], BF, tag="rcb", name=f"rcb{h}")
                        nc.vector.tensor_copy(rcb, rc)
                        bct = ps_mix.tile([128, 512], F32, tag="mix",
                                          name=f"bct{h}")
                        bc = bct[0:64, :]
                        nc.tensor.matmul(bc, ones_bf, rcb, start=True, stop=True)
                        bc_sb = small.tile([64, 512], F32, tag="bcs",
                                           name=f"bcs{h}")
                        nc.vector.tensor_copy(bc_sb, bc)
                        hview = hr[h].rearrange("p (j r) -> p j r", j=16)[
                            :, :, 32 * qb : 32 * (qb + 1)
                        ]
                        nc.vector.tensor_mul(hview, pv[h][0:64, :], bc_sb)
                        # final block: scatter each head as soon as its last
                        # normalize lands, so X^T assembly overlaps the other
                        # head's normalize chain
                        if FILLER and hp == 1 and qb == 3:
                            xh_done[h] = emit_scatter(h, hr[h])
                    # after the last qb of pair 0: queue scatter + WO as filler
                    if FILLER and hp == 0 and qb == 3:
                        for i, h in enumerate((hA, hB)):
                            xh_done[h] = emit_scatter(h, hr[h])
                            filler[(1, 2 + i)].extend([
                                lambda h=h: emit_wo_n(h, xh_done[h], 0),
                                lambda h=h: emit_wo_n(h, xh_done[h], 1),
                            ])

            # tail: remaining heads — scatters first so the second head's
            # X^T assembly overlaps the first head's WO matmuls
            tail_heads = (0, 1, 2, 3) if not FILLER else (2, 3)
            xh_t = {
                h: xh_done.get(h) or emit_scatter(h, hr[h]) for h in tail_heads
            }
            for n in range(2):
                for h in tail_heads:
                    emit_wo_n(h, xh_t[h], n)

    nc.finalize()
    return nc


_QPERM = None


def _qperm():
    """Permute q columns j-major within each 512 block: position j*32+r holds
    original offset r*16+j.  Makes the normalize write into hr's (j r) layout
    contiguous; everything downstream of the scores rhs follows the permuted
    order consistently, and the output mapping is unchanged."""
    global _QPERM
    if _QPERM is None:
        p = np.arange(512)
        perm = (p % 32) * 16 + p // 32
        _QPERM = np.concatenate([512 * qb + perm for qb in range(4)])
    return _QPERM


def make_in_maps(Q, K, V, WQ, WK, WV, WO):
    in_maps = []
    wo_full = np.ascontiguousarray(WO.astype(np.float16))
    Qb = Q[:, _qperm(), :].astype(np.float16)
    Kb = K.astype(np.float16)
    Vb = V.astype(np.float16)
    for b in range(B):
        qTb = np.ascontiguousarray(Qb[b].T)
        kTb = np.ascontiguousarray(Kb[b].T)
        vTb = np.ascontiguousarray(Vb[b].T)
        for g in range(GROUPS):
            hs = slice(4 * g, 4 * g + 4)
            # [4, D, dk] -> [D, 4*dk]
            wqc = np.ascontiguousarray(
                WQ[hs].transpose(1, 0, 2).reshape(D, 256).astype(np.float16)
            )
            wkc = np.ascontiguousarray(
                WK[hs].transpose(1, 0, 2).reshape(D, 256).astype(np.float16)
            )
            wvc = np.ascontiguousarray(
                WV[hs].transpose(1, 0, 2).reshape(D, 256).astype(np.float16)
            )
            in_maps.append(
                {"qT": qTb, "kT": kTb, "vT": vTb,
                 "wq": wqc, "wk": wkc, "wv": wvc, "wo": wo_full}
            )
    return in_maps


def run(inputs, **run_kwargs):
    global _cached_nc
    if _cached_nc is None:
        _cached_nc = build_nc()
    in_maps = make_in_maps(**inputs)
    res = run_bass_kernel_spmd(
        _cached_nc, in_maps, core_ids=list(range(8)), **run_kwargs
    )
    full = np.zeros((B, S, D), np.float32)
    for b in range(B):
        for g in range(GROUPS):
            full[b, 512 * g : 512 * (g + 1), :] = res.results[4 * b + g]["out"]
    return full, res


def kernel(**inputs):
    full, _ = run(inputs)
    return full


if __name__ == "__main__":
    rng = np.random.default_rng(0)
    inputs = {
        "Q": rng.standard_normal((B, S, D)).astype(np.float32),
        "K": rng.standard_normal((B, S, D)).astype(np.float32),
        "V": rng.standard_normal((B, S, D)).astype(np.float32),
        "WQ": (rng.uniform(-0.1, 0.1, (H, D, DK))).astype(np.float32),
        "WK": (rng.uniform(-0.1, 0.1, (H, D, DK))).astype(np.float32),
        "WV": (rng.uniform(-0.1, 0.1, (H, D, DK))).astype(np.float32),
        "WO": (rng.uniform(-0.1, 0.1, (H * DK, D))).astype(np.float32),
    }
    out = kernel(**inputs)
    print("kernel out", out.shape, out.dtype, float(np.abs(out).max()))
